# revision 1
# baseline (speedup 1.0000x reference)
"""Self-contained Trainium2 Bass kernel for nn_Attention_35433480192669.

Windowed multi-head attention: x(4096,16,512) -> roll -> qkv -> 16-head
16-token windowed attention with rel-pos bias + shifted-window mask -> proj.

Sharding: data-parallel over windows, 8 cores x 512 windows.
Device layout: tiles of 128 tokens (8 windows). All matmuls in float32r.
"""
import sys
import dataclasses

sys.path.insert(0, "/opt/trn_rl_repo")
import numpy as np
import concourse.bacc as bacc
import concourse.mybir as mybir
from concourse import tile
from concourse.bass_utils import run_bass_kernel_spmd

# problem constants (hardcoded per spec)
B = 4096          # windows
N = 16            # tokens per window
DIM = 512
HEADS = 16
DH = 64
INNER = HEADS * DH  # 1024
LEN = 4
CORES = 8
BC = B // CORES   # 512 windows / core
T = BC * N        # 8192 tokens / core
TP = 128          # tokens per tile (8 windows)
NT = T // TP      # 64 tiles
G = 4             # tiles per group
NG = NT // G      # 16 groups
KC = DIM // 128   # 4 contraction chunks for x
SCALE = DH ** -0.5
NEG = -1e9
STAGE = None  # debug: 'qk','v','exp','norm','av'

F32 = mybir.dt.float32
F32R = mybir.dt.float32r
BF16 = mybir.dt.bfloat16


def _mask_and_bias(rel_pos):
    """(HEADS,128,128) additive bias B~T[h][j,i] (keys j on axis 1)."""
    # reference mask (16 heads, 16, 16), True = masked
    h, w, p = HEADS // 2, 2, LEN
    s = p - LEN // 2
    m = np.zeros((h, w, p, p, p, p), dtype=bool)
    m[-1, :, :s, :, s:, :] = True
    m[-1, :, s:, :, :s, :] = True
    m[:, -1, :, :s, :, s:] = True
    m[:, -1, :, s:, :, :s] = True
    m = m.reshape(h * w, p * p, p * p)  # (16, pi, pj)

    cord = np.array([[i, j] for i in range(p) for j in range(p)])
    rel = cord[:, None, :] - cord[None, :, :] + p - 1
    r0, r1 = rel[..., 0], rel[..., 1]          # (16,16) indices
    bias = rel_pos[:, r0, r1]                   # (HEADS, pi, pj)
    bias = np.where(m, NEG, bias)               # masked within window

    out = np.full((HEADS, TP, TP), NEG, dtype=np.float32)
    pi = np.arange(TP) % N
    pj = np.arange(TP) % N
    wi = np.arange(TP) // N
    wj = np.arange(TP) // N
    same = (wi[None, :] == wj[:, None])         # (j, i) same-window
    for hh in range(HEADS):
        bt = bias[hh][pi[None, :].repeat(TP, 0), pj[:, None].repeat(TP, 1)]
        # bt[j, i] = bias[h, pi(i), pj(j)]
        out[hh] = np.where(same, bt, NEG)
    return out.astype(np.float32)


def _prep(x, w_qkv, b_qkv, w_proj, b_proj, rel_pos):
    x = np.asarray(x, np.float32)
    w_qkv = np.asarray(w_qkv, np.float32)
    b_qkv = np.asarray(b_qkv, np.float32)
    w_proj = np.asarray(w_proj, np.float32)
    b_proj = np.asarray(b_proj, np.float32)
    rel_pos = np.asarray(rel_pos, np.float32)

    xr = np.roll(x, -(N // 2), axis=1)                    # (B, N, DIM)
    xr = xr.reshape(CORES, BC * N, DIM)                   # per-core tokens

    # x packed: per core (NG, 128p, KC, G, 128t):
    # [g, p, c, u, t] = xT[128c+p, (g*G+u)*128 + t]
    xp = xr.reshape(CORES, NG, G, TP, KC, 128).transpose(0, 1, 5, 4, 2, 3)
    xp = np.ascontiguousarray(xp)

    w_q = w_qkv[:INNER] * SCALE
    w_k = w_qkv[INNER:2 * INNER]
    w_v = w_qkv[2 * INNER:]
    b_q = b_qkv[:INNER] * SCALE
    b_v = b_qkv[2 * INNER:]

    # q,k stationary chunks: (128p, 16m, KC, 128f) = W[128m+f, 128kc+p]
    w_qk = np.concatenate([w_q, w_k], 0)                  # (2048, 512)
    w_qk_p = w_qk.reshape(16, 128, KC, 128).transpose(3, 0, 2, 1)
    w_qk_p = np.ascontiguousarray(w_qk_p)

    # v moving: (128p, KC, 1024f) = w_v[f, 128kc+p]
    w_v_p = w_v.T.reshape(KC, 128, INNER).transpose(1, 0, 2)
    w_v_p = np.ascontiguousarray(w_v_p)

    # proj moving: (128p, 8kc, 512od) = w_proj[od, 128kc+p]
    w_pT = w_proj.T.reshape(8, 128, DIM).transpose(1, 0, 2)
    w_pT = np.ascontiguousarray(w_pT)

    bq_cols = np.zeros((128, 8, 2), np.float32)   # masked per parity
    bqm = b_q.reshape(8, 128).T                    # (128, 8)
    bq_cols[:64, :, 0] = bqm[:64]
    bq_cols[64:, :, 1] = bqm[64:]
    bq_cols = np.ascontiguousarray(bq_cols)
    pmask = np.zeros((128, 2), np.float32)
    pmask[:64, 0] = 1.0
    pmask[64:, 1] = 1.0
    b_adj = b_proj + w_proj @ b_v                                  # (512,)
    bproj_bc = np.ascontiguousarray(np.broadcast_to(b_adj, (128, DIM)))

    biasT = _mask_and_bias(rel_pos)                                # (16,128,128)
    biasT = np.ascontiguousarray(biasT.transpose(1, 0, 2))         # (128j,16h,128i)

    ones32 = np.ones((128, 128), np.float32)
    return xp, w_qk_p, w_v_p, w_pT, bq_cols, pmask, bproj_bc, biasT, ones32


def _build():
    nc = bacc.Bacc("TRN2", target_bir_lowering=False, debug=False,
                   num_devices=CORES)
    d_x = nc.dram_tensor("xp", [NG, TP, KC, G, 128], F32, kind="ExternalInput")
    d_wqk = nc.dram_tensor("w_qk", [128, 16, KC, 128], F32, kind="ExternalInput")
    d_wv = nc.dram_tensor("w_v", [128, KC, INNER], F32, kind="ExternalInput")
    d_wp = nc.dram_tensor("w_pT", [128, 8, DIM], F32, kind="ExternalInput")
    d_bq = nc.dram_tensor("bq", [128, 8, 2], F32, kind="ExternalInput")
    d_pm = nc.dram_tensor("pmask", [128, 2], F32, kind="ExternalInput")
    d_bp = nc.dram_tensor("bproj", [128, DIM], F32, kind="ExternalInput")
    d_bias = nc.dram_tensor("biasT", [128, 16, 128], F32, kind="ExternalInput")
    d_ones = nc.dram_tensor("ones32", [128, 128], F32, kind="ExternalInput")
    d_out = nc.dram_tensor("out", [NT, TP, DIM], F32, kind="ExternalOutput")

    r = F32R
    with tile.TileContext(nc) as tc:
        with tc.tile_pool(name="const", bufs=1) as pc, \
             tc.tile_pool(name="x", bufs=2) as px, \
             tc.tile_pool(name="qk", bufs=16) as pqk, \
             tc.tile_pool(name="vs", bufs=G) as pvs, \
             tc.tile_pool(name="attn", bufs=9) as pat, \
             tc.tile_pool(name="sm", bufs=2) as psm, \
             tc.tile_pool(name="ao", bufs=4) as pao, \
             tc.tile_pool(name="fo", bufs=2) as pfo, \
             tc.tile_pool(name="psqd", bufs=4, space="PSUM") as ppqd, \
             tc.tile_pool(name="pssv", bufs=2, space="PSUM") as ppsv:

            wqk = pc.tile([128, 16, KC, 128], r, tag="wqk")
            wv = pc.tile([128, KC, INNER], r, tag="wv")
            wp = pc.tile([128, 8, DIM], r, tag="wp")
            bq = pc.tile([128, 8, 2], F32, tag="bq")
            pm = pc.tile([128, 2], F32, tag="pm")
            bp = pc.tile([128, DIM], F32, tag="bp")
            bias = pc.tile([128, 16, 128], F32, tag="bias")
            ones = pc.tile([128, 128], r, tag="ones")
            nc.sync.dma_start(out=bias[:], in_=d_bias.ap())
            nc.sync.dma_start(out=bq[:], in_=d_bq.ap())
            nc.sync.dma_start(out=pm[:], in_=d_pm[:, :])
            nc.sync.dma_start(out=ones[:], in_=d_ones.ap().bitcast(r))
            for m in range(16):
                nc.sync.dma_start(out=wqk[:, m], in_=d_wqk.ap().bitcast(r)[:, m])
            for c in range(KC):
                nc.sync.dma_start(out=wv[:, c], in_=d_wv.ap().bitcast(r)[:, c])
            for kc in range(8):
                nc.sync.dma_start(out=wp[:, kc], in_=d_wp.ap().bitcast(r)[:, kc])
            nc.sync.dma_start(out=bp[:], in_=d_bp[:, :])

            state = [None] * G  # per-tile pipeline state

            def gemms(g):
                xt = px.tile([128, KC, G, 128], r, tag="x", bufs=2,
                             name=f"xt{g}")
                nc.sync.dma_start(out=xt[:], in_=d_x.ap().bitcast(r)[g])
                qks = []
                for m in range(16):
                    pq = ppqd.tile([128, 512], F32, tag="qd")
                    for c in range(KC):
                        nc.tensor.matmul(
                            pq[:], wqk[:, m, c, :], xt[:, c, :, :],
                            start=(c == 0), stop=(c == KC - 1))
                    if m < 8:
                        qk = pqk.tile([128, 2, 512], r, tag="qk", bufs=8,
                                      name=f"qk{m}")
                        for par in range(2):
                            nc.vector.tensor_scalar(
                                qk[:, par, :], pq[:],
                                pm[:, par:par + 1], bq[:, m, par:par + 1],
                                mybir.AluOpType.mult, mybir.AluOpType.add)
                        qks.append(qk)
                    else:
                        qk = pqk.tile([128, 512], r, tag="kk", bufs=8,
                                      name=f"kk{m}")
                        nc.scalar.copy(qk[:], pq[:])
                        qks.append(qk)
                vss = []
                for u in range(G):
                    vt = pvs.tile([128, 16, 128], BF16, tag="vs")
                    nc.gpsimd.memset(vt[:], 0.0)
                    for half in range(2):
                        pv = ppqd.tile([128, 512], F32, tag="qd")
                        for c in range(KC):
                            nc.tensor.matmul(
                                pv[:], xt[:, c, u, :],
                                wv[:, c, half * 512:(half + 1) * 512],
                                start=(c == 0), stop=(c == KC - 1))
                        vta = vt[:]
                        dst = dataclasses.replace(
                            vta, offset=vta.offset + 1024 * half,
                            ap=[vta.ap[0], [256, 4], [192, 2], [1, 64]])
                        nc.scalar.copy(dst, pv[:])
                    vss.append(vt)
                return qks, vss

            def front(g, u, qks):
                ps_a = ppsv.tile([128, 1024], F32, tag="sv")
                ps_b = ppsv.tile([128, 1024], F32, tag="sv")
                pss = [ps_a, ps_b]
                ans = []
                for q in range(4):
                    pd = ppqd.tile([128, 512], F32, tag="qd")
                    nc.scalar.copy(pd[:], bias[:, 4 * q:4 * q + 4, :])
                    for mm in range(2):
                        m = 2 * q + mm
                        nc.tensor.matmul(
                            pd[:, mm * 256:mm * 256 + 256],
                            qks[8 + m][:, u * 128:(u + 1) * 128],
                            qks[m][:, :, u * 128:(u + 1) * 128],
                            start=False, stop=True,
                            skip_group_check=True)
                    at = pat.tile([128, 512], r, tag="attn")
                    nc.scalar.activation(at[:], pd[:],
                                         mybir.ActivationFunctionType.Exp)
                    nc.tensor.matmul(pss[q // 2][:, 512 * (q % 2):
                                                 512 * (q % 2) + 512],
                                     ones[:], at[:], start=True, stop=True)
                    ans.append(at)
                return pss, ans

            def back(g, u, vss, pss, ans):
                ub_a = psm.tile([128, 1024], F32, tag="sm", bufs=2)
                nc.vector.reciprocal_approx_fast(out=ub_a[:], in_=pss[0][:])
                ub_b = psm.tile([128, 1024], F32, tag="smb", bufs=2)
                nc.vector.reciprocal_approx_fast(out=ub_b[:], in_=pss[1][:])
                ubs = [ub_a, ub_b]
                av0 = ppqd.tile([128, 512], F32, tag="qd")
                av1 = ppqd.tile([128, 512], F32, tag="qd")
                avs_ = [av0, av1]
                for q in range(4):
                    an = pat.tile([128, 512], BF16, tag="attn_n", bufs=4)
                    nc.vector.tensor_mul(
                        an[:], ans[q][:],
                        ubs[q // 2][:, 512 * (q % 2):
                                    512 * (q % 2) + 512].bitcast(r))
                    for c4 in range(4):
                        h = 4 * q + c4
                        nc.tensor.matmul(
                            avs_[h // 8][:, ((h // 2) % 4) * 128:
                                         ((h // 2) % 4) * 128 + 128],
                            vss[u][:, h, :],
                            an[:, c4 * 128:(c4 + 1) * 128],
                            start=(h % 8 == 0), stop=(h % 8 == 7),
                            skip_group_check=True)
                aos = []
                for b_ in range(2):
                    ao = pao.tile([128, 512], r, tag="ao")
                    nc.scalar.copy(ao[:], avs_[b_][:])
                    aos.append(ao)
                pf = ppqd.tile([128, 512], F32, tag="qd")
                for kc in range(8):
                    nc.tensor.matmul(
                        pf[:],
                        aos[kc // 4][:, (kc % 4) * 128:(kc % 4) * 128 + 128],
                        wp[:, kc, :],
                        start=(kc == 0), stop=(kc == 7))
                f = pfo.tile([128, DIM], F32, tag="fo")
                nc.vector.tensor_add(f[:], pf[:], bp[:])
                nc.sync.dma_start(out=d_out[g * G + u], in_=f[:])

            # software pipeline: front(u+1) emitted before back(u)
            pending = None  # (g, u, vss, pss, ans)
            for g in range(NG):
                qks, vss = gemms(g)
                for u in range(G):
                    fr = front(g, u, qks)
                    if pending is not None:
                        back(*pending)
                    pending = (g, u, vss, fr[0], fr[1])
            back(*pending)
    nc.compile()
    return nc


_NC = None


def kernel(x, w_qkv, b_qkv, w_proj, b_proj, rel_pos, **_):
    global _NC
    xp, w_qk_p, w_v_p, w_pT, bq_cols, pmask, bproj_bc, biasT, ones32 = _prep(
        x, w_qkv, b_qkv, w_proj, b_proj, rel_pos)
    if _NC is None:
        _NC = _build()
    shared = {"w_qk": w_qk_p, "w_v": w_v_p, "w_pT": w_pT, "bq": bq_cols,
              "pmask": pmask, "bproj": bproj_bc, "biasT": biasT,
              "ones32": ones32}
    in_maps = [dict(shared, xp=np.ascontiguousarray(xp[c]))
               for c in range(CORES)]
    res = run_bass_kernel_spmd(_NC, in_maps, list(range(CORES)))
    outs = [res.results[c]["out"].reshape(T, DIM) for c in range(CORES)]
    return np.concatenate(outs, 0).reshape(B, N, DIM)



# revision 2
# speedup vs baseline: 3.0167x; 3.0167x over previous
"""Self-contained Trainium2 Bass kernel for nn_Attention_35433480192669.

Windowed multi-head attention: x(4096,16,512) -> roll -> qkv -> 16-head
16-token windowed attention with rel-pos bias + shifted-window mask -> proj.

Sharding: data-parallel over windows, 8 cores x 512 windows.
Device layout: tiles of 128 tokens (8 windows). Matmuls in bf16 with f32
accumulate; all wire traffic (x, weights, output) is bf16 to halve the
host<->device transfer volume, which dominates wall time under axon.
"""
import sys
import dataclasses

sys.path.insert(0, "/opt/trn_rl_repo")
import numpy as np
import ml_dtypes
import concourse.bacc as bacc
import concourse.mybir as mybir
from concourse import tile
from concourse.bass_utils import run_bass_kernel_spmd

# problem constants (hardcoded per spec)
B = 4096          # windows
N = 16            # tokens per window
DIM = 512
HEADS = 16
DH = 64
INNER = HEADS * DH  # 1024
LEN = 4
CORES = 8
BC = B // CORES   # 512 windows / core
T = BC * N        # 8192 tokens / core
TP = 128          # tokens per tile (8 windows)
NT = T // TP      # 64 tiles
G = 4             # tiles per group
NG = NT // G      # 16 groups
KC = DIM // 128   # 4 contraction chunks for x
SCALE = DH ** -0.5
NEG = -1e9

F32 = mybir.dt.float32
BF16 = mybir.dt.bfloat16
NPBF16 = ml_dtypes.bfloat16


def _mask_and_bias(rel_pos):
    """(HEADS,128,128) additive bias B~T[h][j,i] (keys j on axis 1)."""
    # reference mask (16 heads, 16, 16), True = masked
    h, w, p = HEADS // 2, 2, LEN
    s = p - LEN // 2
    m = np.zeros((h, w, p, p, p, p), dtype=bool)
    m[-1, :, :s, :, s:, :] = True
    m[-1, :, s:, :, :s, :] = True
    m[:, -1, :, :s, :, s:] = True
    m[:, -1, :, s:, :, :s] = True
    m = m.reshape(h * w, p * p, p * p)  # (16, pi, pj)

    cord = np.array([[i, j] for i in range(p) for j in range(p)])
    rel = cord[:, None, :] - cord[None, :, :] + p - 1
    r0, r1 = rel[..., 0], rel[..., 1]          # (16,16) indices
    bias = rel_pos[:, r0, r1]                   # (HEADS, pi, pj)
    bias = np.where(m, NEG, bias)               # masked within window

    out = np.full((HEADS, TP, TP), NEG, dtype=np.float32)
    pi = np.arange(TP) % N
    pj = np.arange(TP) % N
    wi = np.arange(TP) // N
    wj = np.arange(TP) // N
    same = (wi[None, :] == wj[:, None])         # (j, i) same-window
    for hh in range(HEADS):
        bt = bias[hh][pi[None, :].repeat(TP, 0), pj[:, None].repeat(TP, 1)]
        # bt[j, i] = bias[h, pi(i), pj(j)]
        out[hh] = np.where(same, bt, NEG)
    return out.astype(np.float32)


def _prep(x, w_qkv, b_qkv, w_proj, b_proj, rel_pos):
    w_qkv = np.asarray(w_qkv, np.float32)
    b_qkv = np.asarray(b_qkv, np.float32)
    w_proj = np.asarray(w_proj, np.float32)
    b_proj = np.asarray(b_proj, np.float32)
    rel_pos = np.asarray(rel_pos, np.float32)

    xb = np.asarray(x).astype(NPBF16)                     # (B, N, DIM) bf16
    xr = np.roll(xb, -(N // 2), axis=1)                   # cyclic shift
    xr = xr.reshape(CORES, BC * N, DIM)                   # per-core tokens

    # x packed: per core (NG, 128p, KC, G, 128t):
    # [g, p, c, u, t] = xT[128c+p, (g*G+u)*128 + t]
    xp = xr.reshape(CORES, NG, G, TP, KC, 128).transpose(0, 1, 5, 4, 2, 3)
    xp = np.ascontiguousarray(xp)

    w_q = w_qkv[:INNER] * SCALE
    w_k = w_qkv[INNER:2 * INNER]
    w_v = w_qkv[2 * INNER:]
    b_q = b_qkv[:INNER] * SCALE
    b_v = b_qkv[2 * INNER:]

    # q,k stationary chunks: (128p, 16m, KC, 128f) = W[128m+f, 128kc+p]
    w_qk = np.concatenate([w_q, w_k], 0)                  # (2048, 512)
    w_qk_p = w_qk.reshape(16, 128, KC, 128).transpose(3, 0, 2, 1)
    w_qk_p = np.ascontiguousarray(w_qk_p.astype(NPBF16))

    # v moving: (128p, KC, 1024f) = w_v[f, 128kc+p]
    w_v_p = w_v.T.reshape(KC, 128, INNER).transpose(1, 0, 2)
    w_v_p = np.ascontiguousarray(w_v_p.astype(NPBF16))

    # proj moving: (128p, 8kc, 512od) = w_proj[od, 128kc+p]
    w_pT = w_proj.T.reshape(8, 128, DIM).transpose(1, 0, 2)
    w_pT = np.ascontiguousarray(w_pT.astype(NPBF16))

    bq_cols = np.zeros((128, 8, 2), np.float32)   # masked per parity
    bqm = b_q.reshape(8, 128).T                    # (128, 8)
    bq_cols[:64, :, 0] = bqm[:64]
    bq_cols[64:, :, 1] = bqm[64:]
    bq_cols = np.ascontiguousarray(bq_cols)
    pmask = np.zeros((128, 2), np.float32)
    pmask[:64, 0] = 1.0
    pmask[64:, 1] = 1.0
    b_adj = b_proj + w_proj @ b_v                                  # (512,)
    bproj_bc = np.ascontiguousarray(np.broadcast_to(b_adj, (128, DIM)))

    biasT = _mask_and_bias(rel_pos)                                # (16,128,128)
    biasT = np.ascontiguousarray(
        biasT.transpose(1, 0, 2).astype(NPBF16))                   # (128j,16h,128i)

    ones32 = np.ones((128, 128), NPBF16)
    return xp, w_qk_p, w_v_p, w_pT, bq_cols, pmask, bproj_bc, biasT, ones32


def _build():
    nc = bacc.Bacc("TRN2", target_bir_lowering=False, debug=False,
                   num_devices=CORES)
    d_x = nc.dram_tensor("xp", [NG, TP, KC, G, 128], BF16, kind="ExternalInput")
    d_wqk = nc.dram_tensor("w_qk", [128, 16, KC, 128], BF16, kind="ExternalInput")
    d_wv = nc.dram_tensor("w_v", [128, KC, INNER], BF16, kind="ExternalInput")
    d_wp = nc.dram_tensor("w_pT", [128, 8, DIM], BF16, kind="ExternalInput")
    d_bq = nc.dram_tensor("bq", [128, 8, 2], F32, kind="ExternalInput")
    d_pm = nc.dram_tensor("pmask", [128, 2], F32, kind="ExternalInput")
    d_bp = nc.dram_tensor("bproj", [128, DIM], F32, kind="ExternalInput")
    d_bias = nc.dram_tensor("biasT", [128, 16, 128], BF16, kind="ExternalInput")
    d_ones = nc.dram_tensor("ones32", [128, 128], BF16, kind="ExternalInput")
    d_out = nc.dram_tensor("out", [NT, TP, DIM], BF16, kind="ExternalOutput")

    with tile.TileContext(nc) as tc:
        with tc.tile_pool(name="const", bufs=1) as pc, \
             tc.tile_pool(name="x", bufs=2) as px, \
             tc.tile_pool(name="qk", bufs=16) as pqk, \
             tc.tile_pool(name="vs", bufs=G) as pvs, \
             tc.tile_pool(name="attn", bufs=9) as pat, \
             tc.tile_pool(name="sm", bufs=2) as psm, \
             tc.tile_pool(name="ao", bufs=4) as pao, \
             tc.tile_pool(name="fo", bufs=2) as pfo, \
             tc.tile_pool(name="psqd", bufs=4, space="PSUM") as ppqd, \
             tc.tile_pool(name="pssv", bufs=2, space="PSUM") as ppsv:

            wqk = pc.tile([128, 16, KC, 128], BF16, tag="wqk")
            wv = pc.tile([128, KC, INNER], BF16, tag="wv")
            wp = pc.tile([128, 8, DIM], BF16, tag="wp")
            bq = pc.tile([128, 8, 2], F32, tag="bq")
            pm = pc.tile([128, 2], F32, tag="pm")
            bp = pc.tile([128, DIM], F32, tag="bp")
            bias = pc.tile([128, 16, 128], BF16, tag="bias")
            ones = pc.tile([128, 128], BF16, tag="ones")
            nc.sync.dma_start(out=bias[:], in_=d_bias.ap())
            nc.sync.dma_start(out=bq[:], in_=d_bq.ap())
            nc.sync.dma_start(out=pm[:], in_=d_pm[:, :])
            nc.sync.dma_start(out=ones[:], in_=d_ones.ap())
            for m in range(16):
                nc.sync.dma_start(out=wqk[:, m], in_=d_wqk.ap()[:, m])
            for c in range(KC):
                nc.sync.dma_start(out=wv[:, c], in_=d_wv.ap()[:, c])
            for kc in range(8):
                nc.sync.dma_start(out=wp[:, kc], in_=d_wp.ap()[:, kc])
            nc.sync.dma_start(out=bp[:], in_=d_bp[:, :])

            def gemms(g):
                xt = px.tile([128, KC, G, 128], BF16, tag="x", bufs=2,
                             name=f"xt{g}")
                nc.sync.dma_start(out=xt[:], in_=d_x.ap()[g])
                qks = []
                for m in range(16):
                    pq = ppqd.tile([128, 512], F32, tag="qd")
                    for c in range(KC):
                        nc.tensor.matmul(
                            pq[:], wqk[:, m, c, :], xt[:, c, :, :],
                            start=(c == 0), stop=(c == KC - 1))
                    if m < 8:
                        qk = pqk.tile([128, 2, 512], BF16, tag="qk", bufs=8,
                                      name=f"qk{m}")
                        for par in range(2):
                            nc.vector.tensor_scalar(
                                qk[:, par, :], pq[:],
                                pm[:, par:par + 1], bq[:, m, par:par + 1],
                                mybir.AluOpType.mult, mybir.AluOpType.add)
                        qks.append(qk)
                    else:
                        qk = pqk.tile([128, 512], BF16, tag="kk", bufs=8,
                                      name=f"kk{m}")
                        nc.scalar.copy(qk[:], pq[:])
                        qks.append(qk)
                vss = []
                for u in range(G):
                    vt = pvs.tile([128, 16, 128], BF16, tag="vs")
                    nc.gpsimd.memset(vt[:], 0.0)
                    for half in range(2):
                        pv = ppqd.tile([128, 512], F32, tag="qd")
                        for c in range(KC):
                            nc.tensor.matmul(
                                pv[:], xt[:, c, u, :],
                                wv[:, c, half * 512:(half + 1) * 512],
                                start=(c == 0), stop=(c == KC - 1))
                        vta = vt[:]
                        dst = dataclasses.replace(
                            vta, offset=vta.offset + 1024 * half,
                            ap=[vta.ap[0], [256, 4], [192, 2], [1, 64]])
                        nc.scalar.copy(dst, pv[:])
                    vss.append(vt)
                return qks, vss

            def front(g, u, qks):
                ps_a = ppsv.tile([128, 1024], F32, tag="sv")
                ps_b = ppsv.tile([128, 1024], F32, tag="sv")
                pss = [ps_a, ps_b]
                ans = []
                for q in range(4):
                    pd = ppqd.tile([128, 512], F32, tag="qd")
                    nc.scalar.copy(pd[:], bias[:, 4 * q:4 * q + 4, :])
                    for mm in range(2):
                        m = 2 * q + mm
                        nc.tensor.matmul(
                            pd[:, mm * 256:mm * 256 + 256],
                            qks[8 + m][:, u * 128:(u + 1) * 128],
                            qks[m][:, :, u * 128:(u + 1) * 128],
                            start=False, stop=True,
                            skip_group_check=True)
                    at = pat.tile([128, 512], BF16, tag="attn")
                    nc.scalar.activation(at[:], pd[:],
                                         mybir.ActivationFunctionType.Exp)
                    nc.tensor.matmul(pss[q // 2][:, 512 * (q % 2):
                                                 512 * (q % 2) + 512],
                                     ones[:], at[:], start=True, stop=True)
                    ans.append(at)
                return pss, ans

            def back(g, u, vss, pss, ans):
                ub_a = psm.tile([128, 1024], F32, tag="sm", bufs=2)
                nc.vector.reciprocal_approx_fast(out=ub_a[:], in_=pss[0][:])
                ub_b = psm.tile([128, 1024], F32, tag="smb", bufs=2)
                nc.vector.reciprocal_approx_fast(out=ub_b[:], in_=pss[1][:])
                ubs = [ub_a, ub_b]
                av0 = ppqd.tile([128, 512], F32, tag="qd")
                av1 = ppqd.tile([128, 512], F32, tag="qd")
                avs_ = [av0, av1]
                for q in range(4):
                    an = pat.tile([128, 512], BF16, tag="attn_n", bufs=4)
                    nc.vector.tensor_mul(
                        an[:], ans[q][:],
                        ubs[q // 2][:, 512 * (q % 2):512 * (q % 2) + 512])
                    for c4 in range(4):
                        h = 4 * q + c4
                        nc.tensor.matmul(
                            avs_[h // 8][:, ((h // 2) % 4) * 128:
                                         ((h // 2) % 4) * 128 + 128],
                            vss[u][:, h, :],
                            an[:, c4 * 128:(c4 + 1) * 128],
                            start=(h % 8 == 0), stop=(h % 8 == 7),
                            skip_group_check=True)
                aos = []
                for b_ in range(2):
                    ao = pao.tile([128, 512], BF16, tag="ao")
                    nc.scalar.copy(ao[:], avs_[b_][:])
                    aos.append(ao)
                pf = ppqd.tile([128, 512], F32, tag="qd")
                for kc in range(8):
                    nc.tensor.matmul(
                        pf[:],
                        aos[kc // 4][:, (kc % 4) * 128:(kc % 4) * 128 + 128],
                        wp[:, kc, :],
                        start=(kc == 0), stop=(kc == 7))
                f = pfo.tile([128, DIM], BF16, tag="fo")
                nc.vector.tensor_add(f[:], pf[:], bp[:])
                nc.sync.dma_start(out=d_out[g * G + u], in_=f[:])

            # software pipeline: front(u+1) emitted before back(u)
            pending = None  # (g, u, vss, pss, ans)
            for g in range(NG):
                qks, vss = gemms(g)
                for u in range(G):
                    fr = front(g, u, qks)
                    if pending is not None:
                        back(*pending)
                    pending = (g, u, vss, fr[0], fr[1])
            back(*pending)
    nc.compile()
    return nc


_NC = None


def kernel(x, w_qkv, b_qkv, w_proj, b_proj, rel_pos, **_):
    global _NC
    xp, w_qk_p, w_v_p, w_pT, bq_cols, pmask, bproj_bc, biasT, ones32 = _prep(
        x, w_qkv, b_qkv, w_proj, b_proj, rel_pos)
    if _NC is None:
        _NC = _build()
    shared = {"w_qk": w_qk_p, "w_v": w_v_p, "w_pT": w_pT, "bq": bq_cols,
              "pmask": pmask, "bproj": bproj_bc, "biasT": biasT,
              "ones32": ones32}
    in_maps = [dict(shared, xp=xp[c]) for c in range(CORES)]
    res = run_bass_kernel_spmd(_NC, in_maps, list(range(CORES)))
    outs = [res.results[c]["out"].reshape(T, DIM) for c in range(CORES)]
    full = np.concatenate(outs, 0).astype(np.float32)
    return full.reshape(B, N, DIM)
